# revision 34
# baseline (speedup 1.0000x reference)
"""Trainium2 Bass kernel for sliding-window attention layer.

Sharding: DP over batch (2) x TP over head groups (4) = 8 cores.
Core c handles batch b=c//4, head group hg=c%4 (4 heads, 512 features).

All-bf16 datapath, no DRAM spills, engine-balanced:
  Phase 1: q/k feature-major GEMM (w stationary, 2-chunk token bursts,
           chunked w/hs loads so the first MMs start after ~6MB);
           sumsq squared on ACT straight from PSUM, reduced via
           ones-matmul, one fused AllReduce [1,4096] per 4-core group.
           RoPE (raw cos/sin, no norm factor) + v token-major GEMM
           overlap the collective.  PSUM->SBUF copies on DVE; ACT does
           only Square/Sqrt/Exp (3 table loads per rep).
  Phase 2: rsqrt factors broadcast via ones-matmul, applied
           multiplicatively to rope'd q/k (q side also carries
           1/sqrt(HD)).  Attention per (qc, head): s^T blocks
           [128k, 512q]; window masks injected on the PE (identity
           matmul accumulate, -30000 bf16), exp on ACT, PV accumulate,
           denominator via ones-matmul batched after the PV stream
           (ones_col stays stationary), reciprocal mult.  After each
           token chunk's 4 heads: output projection for that chunk
           (interleaves PE proj with next chunk's attention).
Host: sum 4 bf16 partials per batch in f32, transpose back.

`reps` replicates the body N times for amortized device-time
measurement (test.py differences two rep counts).
"""
import os
from contextlib import ExitStack

import numpy as np
import ml_dtypes

import concourse.bass as bass
import concourse.mybir as mybir
import concourse.tile as tile
from concourse import bacc
from concourse.bass_utils import run_bass_kernel_spmd

H = 2048
NH = 16
HD = 128
WINDOW = 1024
EPS = 1e-6
THETA = 10000.0
B = 2
S = 2048
HG = 4            # head groups (TP degree)
HPG = NH // HG    # heads per group
FPG = HPG * HD    # features per group (512)
N_CORES = 8
P = 128
TC = 512          # token chunk (matmul free dim)
NTC = S // TC     # 4
NET = H // P      # 16 contraction tiles

f32 = mybir.dt.float32
f32r = mybir.dt.float32r
bf16 = mybir.dt.bfloat16
AF = mybir.ActivationFunctionType
ALU = mybir.AluOpType

_cache = {}
ABLATE = {}


def _host_consts():
    pos = np.arange(S, dtype=np.float64)
    invf = 1.0 / (THETA ** (np.arange(0, HD, 2, dtype=np.float64) / HD))
    ang = invf[:, None] * pos[None, :]
    c64 = np.cos(ang)
    s64 = np.sin(ang)
    cos = np.concatenate([c64, c64], axis=0).astype(np.float32)   # [128, S]
    sin = np.concatenate([-s64, s64], axis=0).astype(np.float32)  # [-sin; +sin]
    # 8 partial-block mask patterns [128 k, 512 q]; deltas -384..0, 640..1024
    deltas = [-384, -256, -128, 0, 640, 768, 896, 1024]
    masks = np.zeros((8, P, TC), np.float32)
    kk = np.arange(P)[:, None]
    qq = np.arange(TC)[None, :]
    for j, d in enumerate(deltas):
        valid = (d + qq - kk >= 0) & (d + qq - kk <= WINDOW - 1)
        masks[j] = np.where(valid, 0.0, -30000.0)
    masks = masks.astype(ml_dtypes.bfloat16)
    return cos, sin, masks


# valid-column bounds per mask pattern (cols outside are fully masked):
# start-side deltas -384/-256/-128 begin at col -d; end-side 768/896/1024
# end at col 1024-d+128 (rounded up to the pattern's bounding box)
_PAT_COLS = {0: (384, TC), 1: (256, TC), 2: (128, TC), 3: (0, TC),
             4: (0, TC), 5: (0, 384), 6: (0, 256), 7: (0, 128)}


def _kb_list(qc):
    """Valid key blocks for q-chunk qc: (kb, pattern or None, c0, c1).

    Full-width blocks are ordered first so the start=True matmul of each
    PSUM accumulation group initializes every element of the bank before
    any column-restricted block accumulates into a subset.
    """
    out = []
    for kb in range(S // P):
        d = qc * TC - kb * P
        if d < -(TC - P) or d > WINDOW:
            continue
        if P <= d <= WINDOW - TC:
            out.append((kb, None, 0, TC))
            continue
        pat = d // P + 3 if d <= 0 else d // P - 5 + 4
        c0, c1 = _PAT_COLS[pat]
        out.append((kb, pat, c0, c1))
    out.sort(key=lambda b: b[3] - b[2], reverse=True)
    return out


def _build_ablate(apply_norm_w, reps, key):
    return _build(apply_norm_w, reps, _key=key)


def _build(apply_norm_w, reps=1, _key=None):
    key = _key or ("nc", apply_norm_w, reps)
    if key in _cache:
        return _cache[key]

    nc = bacc.Bacc("TRN2", target_bir_lowering=False, debug=False,
                   num_devices=N_CORES)

    hsT_in = nc.dram_tensor("hsT", [H, S], bf16, kind="ExternalInput").ap()
    wT_in = nc.dram_tensor("wT", [H, 3 * FPG], bf16, kind="ExternalInput").ap()
    woT_in = nc.dram_tensor("woT", [FPG, H], bf16, kind="ExternalInput").ap()
    qw_in = nc.dram_tensor("qw", [FPG], f32, kind="ExternalInput").ap()
    kw_in = nc.dram_tensor("kw", [FPG], f32, kind="ExternalInput").ap()
    out_ext = nc.dram_tensor("outT", [H, S], bf16, kind="ExternalOutput").ap()

    cos_np, sin_np, masks_np = _host_consts()
    cos_d = nc.inline_tensor(cos_np.astype(ml_dtypes.bfloat16),
                             name="cos_c").ap()
    sin_d = nc.inline_tensor(sin_np.astype(ml_dtypes.bfloat16),
                             name="sin_c").ap()
    masks_d = nc.inline_tensor(np.ascontiguousarray(
        masks_np.transpose(1, 0, 2)), name="masks_c").ap()  # [128, 8, 512]
    ident_d = nc.inline_tensor(
        np.eye(P, dtype=ml_dtypes.bfloat16), name="ident_c").ap()
    ones_bf_d = nc.inline_tensor(
        np.ones((P, 1), ml_dtypes.bfloat16), name="onesb_c").ap()
    ones_f_d = nc.inline_tensor(np.ones((1, P), np.float32), name="onesf_c").ap()
    scl_d = nc.inline_tensor(
        np.full((1, P), 1.0 / np.sqrt(HD), np.float32), name="scl_c").ap()

    with tile.TileContext(nc) as tc_:
        with ExitStack() as outer:
            cpool = outer.enter_context(tc_.tile_pool(name="consts", bufs=1))
            dram = outer.enter_context(
                tc_.tile_pool(name="dram", bufs=2, space="DRAM"))

            ones_col = cpool.tile([P, 1], bf16, tag="ones_col")
            nc.sync.dma_start(ones_col[:], ones_bf_d[:, :])
            ones_row = cpool.tile([1, P], f32r, tag="ones_row")
            nc.sync.dma_start(ones_row[:], ones_f_d[:, :].bitcast(f32r))
            scl_row = cpool.tile([1, P], f32r, tag="scl_row")
            nc.sync.dma_start(scl_row[:], scl_d[:].bitcast(f32r))
            ident_sb = cpool.tile([P, P], bf16, tag="ident")
            nc.sync.dma_start(ident_sb[:], ident_d[:, :])
            cos_sb = cpool.tile([P, S], bf16, tag="cos")
            nc.sync.dma_start(cos_sb[:], cos_d[:])
            sin_sb = cpool.tile([P, S], bf16, tag="sin")
            nc.sync.dma_start(sin_sb[:], sin_d[:])
            if apply_norm_w:
                qw_sb = cpool.tile([P, HPG], f32, tag="qw")
                nc.sync.dma_start(qw_sb[:], qw_in.rearrange("(a d) -> d a", d=P))
                kw_sb = cpool.tile([P, HPG], f32, tag="kw")
                nc.sync.dma_start(kw_sb[:], kw_in.rearrange("(a d) -> d a", d=P))

            for _rep in range(reps):
                _emit_body(nc, tc_, hsT_in, wT_in, woT_in, out_ext,
                           dram, ones_col, ones_row, scl_row, ident_sb,
                           masks_d, cos_sb, sin_sb,
                           qw_sb if apply_norm_w else None,
                           kw_sb if apply_norm_w else None,
                           apply_norm_w, _rep)

    nc.compile()
    _cache[key] = nc
    return nc


def _emit_body(nc, tc_, hsT_in, wT_in, woT_in, out_ext, dram,
               ones_col, ones_row, scl_row, ident_sb, masks_d,
               cos_sb, sin_sb, qw_sb, kw_sb, apply_norm_w, rep):
    with ExitStack() as body:
        qkv_pool = body.enter_context(
            tc_.tile_pool(name="qkv%d" % rep, bufs=1))
        qT_sb = qkv_pool.tile([P, HPG, S], bf16, tag="qT")
        kT_sb = qkv_pool.tile([P, HPG, S], bf16, tag="kT")
        v_tm = qkv_pool.tile([P, S // P, FPG], bf16, tag="vtm")
        attn_sb = qkv_pool.tile([P, HPG, S], bf16, tag="attn")

        ar_in = dram.tile([1, 2 * S], f32)
        ar_out = dram.tile([1, 2 * S], f32)

        # ---------------- Phase 1: QKV GEMM ----------------
        with ExitStack() as ph1:
            wpool = ph1.enter_context(tc_.tile_pool(name="w%d" % rep, bufs=1))
            stg = ph1.enter_context(tc_.tile_pool(name="stg%d" % rep, bufs=4))
            swpool = ph1.enter_context(
                tc_.tile_pool(name="sw%d" % rep, bufs=2))
            psA = ph1.enter_context(
                tc_.tile_pool(name="psA%d" % rep, bufs=4, space="PSUM"))
            psV = ph1.enter_context(
                tc_.tile_pool(name="psV%d" % rep, bufs=2, space="PSUM"))
            psQ = ph1.enter_context(
                tc_.tile_pool(name="psQ%d" % rep, bufs=2, space="PSUM"))

            # chunked loads so the first GEMM starts after ~6MB, not 14MB:
            # w_q -> hs half 0 -> hs half 1 -> w_k -> w_v
            w_sb = wpool.tile([P, NET, 3 * FPG], bf16, tag="w")
            hs_sb = wpool.tile([P, NET, S], bf16, tag="hs")
            wT_r = wT_in.rearrange("(et p) f -> p et f", p=P)
            hsT_r = hsT_in.rearrange("(et p) t -> p et t", p=P)
            nc.sync.dma_start(w_sb[:, :, 0:FPG], wT_r[:, :, 0:FPG])
            nc.sync.dma_start(hs_sb[:, :, 0:S // 2], hsT_r[:, :, 0:S // 2])
            nc.sync.dma_start(hs_sb[:, :, S // 2:S], hsT_r[:, :, S // 2:S])
            nc.sync.dma_start(w_sb[:, :, FPG:2 * FPG], wT_r[:, :, FPG:2 * FPG])
            nc.sync.dma_start(w_sb[:, :, 2 * FPG:3 * FPG],
                              wT_r[:, :, 2 * FPG:3 * FPG])

            # q/k feature-major: w stationary, 2-chunk token bursts.
            # The GEMM stream stays pure (no interleaved ssq matmuls);
            # sumsq runs as a separate pass below with ones_col stationary.
            for ft in range(0 if not ABLATE.get("noqk") else 8, 8):
                is_q = ft < 4
                h = ft % 4
                dest = qT_sb if is_q else kT_sb
                for tp in range(2):           # token-chunk pairs
                    mm_ps = [psA.tile([P, TC], f32, tag="mmA",
                                      name="mm%d_%d" % (ft, tp * 2 + t))
                             for t in range(2)]
                    for et in range(NET):
                        for t in range(2):
                            tci = tp * 2 + t
                            nc.tensor.matmul(
                                mm_ps[t][:],
                                w_sb[:, et, ft * P:(ft + 1) * P],
                                hs_sb[:, et, tci * TC:(tci + 1) * TC],
                                start=(et == 0), stop=(et == NET - 1))
                    for t in range(2):
                        tci = tp * 2 + t
                        sl = slice(tci * TC, (tci + 1) * TC)
                        nc.vector.tensor_copy(dest[:, h, sl], mm_ps[t][:])

            # sumsq pass (pre-rope: reads raw bf16 q/k), squares on DVE,
            # ones_col stationary across all 32 reduction matmuls
            for side, src in (() if ABLATE.get("noqk") else
                              ((0, qT_sb), (1, kT_sb))):
                for tci in range(NTC):
                    sl = slice(tci * TC, (tci + 1) * TC)
                    ssq_ps = psQ.tile([1, TC], f32, tag="ssq")
                    for h in range(HPG):
                        sq = stg.tile([P, TC], bf16, tag="sq")
                        nc.vector.tensor_tensor(
                            sq[:], src[:, h, sl], src[:, h, sl], ALU.mult)
                        nc.tensor.matmul(
                            ssq_ps[:], ones_col[:], sq[:],
                            start=(h == 0), stop=(h == HPG - 1))
                    off = (0 if side == 0 else S) + tci * TC
                    sst = stg.tile([1, TC], f32, tag="ssq_st")
                    nc.vector.tensor_copy(sst[:], ssq_ps[:])
                    nc.sync.dma_start(ar_in[0:1, off:off + TC], sst[:])

            if ABLATE.get("nocoll"):
                nc.gpsimd.dma_start(ar_out[:], ar_in[:])
            else:
                nc.gpsimd.collective_compute(
                    "AllReduce", ALU.add,
                    replica_groups=[[0, 1, 2, 3], [4, 5, 6, 7]],
                    ins=[ar_in.opt()], outs=[ar_out.opt()])

            # RoPE on raw q/k (no norm factor needed -> overlaps collective)
            def rope(xv, nm, wsb, h):
                if apply_norm_w:
                    nc.vector.tensor_scalar_mul(xv, xv, wsb[:, h:h + 1])
                xsw = swpool.tile([P, S], bf16, tag="xsw", name="xsw_" + nm)
                nc.sync.dma_start(xsw[0:64, :], xv[64:P, :])
                nc.sync.dma_start(xsw[64:P, :], xv[0:64, :])
                nc.vector.tensor_tensor(xv, xv, cos_sb[:], ALU.mult)
                nc.vector.tensor_tensor(xsw[:], xsw[:], sin_sb[:], ALU.mult)
                nc.vector.tensor_tensor(xv, xv, xsw[:], ALU.add)

            if not ABLATE.get("noqk"):
                for h in range(HPG):
                    rope(qT_sb[:, h, :], "q%d" % h, qw_sb, h)
                    rope(kT_sb[:, h, :], "k%d" % h, kw_sb, h)

            # v token-major (overlaps the collective): lhsT = hs^T tiles
            for tb in range(0 if not ABLATE.get("nov") else S // P, S // P):
                v_ps = psV.tile([P, FPG], f32, tag="vps")
                for et in range(NET):
                    nc.tensor.matmul(
                        v_ps[:],
                        hs_sb[:, et, tb * P:(tb + 1) * P],
                        w_sb[:, et, 2 * FPG:3 * FPG],
                        start=(et == 0), stop=(et == NET - 1))
                if True:
                    nc.vector.tensor_copy(v_tm[:, tb, :], v_ps[:])
                else:
                    nc.scalar.activation(v_tm[:, tb, :], v_ps[:], AF.Copy)

        # ---------------- Phase 2: norm factors + attention ----------------
        with ExitStack() as ph2:
            npool = ph2.enter_context(tc_.tile_pool(name="nf%d" % rep, bufs=1))
            nfq_b = npool.tile([P, S], bf16, tag="nfq")
            nfk_b = npool.tile([P, S], bf16, tag="nfk")

            with ExitStack() as tb_scope:
                tbp = tb_scope.enter_context(
                    tc_.tile_pool(name="tb%d" % rep, bufs=2))
                psN = tb_scope.enter_context(
                    tc_.tile_pool(name="psN%d" % rep, bufs=2, space="PSUM"))
                ssqf = tbp.tile([1, 2 * S], f32, tag="ssqf", name="ssqf")
                nc.sync.dma_start(ssqf[:], ar_out[:])
                for side in range(2):  # 0: q, 1: k
                    for t in range(NTC):
                        off = side * S + t * TC
                        sl = slice(t * TC, (t + 1) * TC)
                        var = tbp.tile([1, TC], f32, tag="var")
                        nc.vector.tensor_scalar(
                            var[:], ssqf[0:1, off:off + TC], 1.0 / H, EPS,
                            ALU.mult, ALU.add)
                        inv = tbp.tile([1, TC], f32, tag="invr")
                        nc.vector.reciprocal(inv[:], var[:])
                        rsc = tbp.tile([1, TC], f32r, tag="rsc")
                        nc.scalar.activation(rsc[:], inv[:], AF.Sqrt)
                        lt = scl_row if side == 0 else ones_row
                        nf_ps = psN.tile([P, TC], f32, tag="nf")
                        nc.tensor.matmul(nf_ps[:], lt[:], rsc[:],
                                         start=True, stop=True)
                        dst = nfq_b if side == 0 else nfk_b
                        if True:
                            nc.vector.tensor_copy(dst[:, sl], nf_ps[:])
                        else:
                            nc.scalar.activation(dst[:, sl], nf_ps[:], AF.Copy)

            # apply norm factors to rope'd q/k
            if not ABLATE.get("noqk"):
                for h in range(HPG):
                    nc.vector.tensor_tensor(
                        qT_sb[:, h, :], qT_sb[:, h, :], nfq_b[:], ALU.mult)
                    nc.vector.tensor_tensor(
                        kT_sb[:, h, :], kT_sb[:, h, :], nfk_b[:], ALU.mult)

            ppool = ph2.enter_context(tc_.tile_pool(name="pp%d" % rep, bufs=4))
            pexp = ph2.enter_context(tc_.tile_pool(name="pe%d" % rep, bufs=13))
            wopool = ph2.enter_context(tc_.tile_pool(name="wo%d" % rep, bufs=1))
            ostg = ph2.enter_context(tc_.tile_pool(name="os%d" % rep, bufs=2))
            psS = ph2.enter_context(
                tc_.tile_pool(name="psS%d" % rep, bufs=2, space="PSUM"))
            psO = ph2.enter_context(
                tc_.tile_pool(name="psO%d" % rep, bufs=2, space="PSUM"))
            psD = ph2.enter_context(
                tc_.tile_pool(name="psD%d" % rep, bufs=1, space="PSUM"))
            psB = ph2.enter_context(
                tc_.tile_pool(name="psB%d" % rep, bufs=1, space="PSUM"))
            psP = ph2.enter_context(
                tc_.tile_pool(name="psP%d" % rep, bufs=2, space="PSUM"))

            wo_sb = wopool.tile([P, HPG, H], bf16, tag="wo")
            nc.sync.dma_start(wo_sb[:],
                              woT_in.rearrange("(ft p) o -> p ft o", p=P))
            masks_sb = wopool.tile([P, 8, TC], bf16, tag="masks")
            nc.sync.dma_start(masks_sb[:], masks_d[:])

            for qc in range(NTC):
                qsl = slice(qc * TC, (qc + 1) * TC)
                blocks = _kb_list(qc)
                for h in range(0 if not ABLATE.get("noattn") else HPG, HPG):
                    kh = kT_sb[:, h, :]
                    out_ps = psO.tile([P, TC], f32, tag="pv")
                    den_ps = psD.tile([1, TC], f32, tag="den")
                    p_tiles = []
                    for i, (kb, pat, c0, c1) in enumerate(blocks):
                        s_ps = psS.tile([P, TC], f32, tag="s")
                        nc.tensor.matmul(
                            s_ps[:, c0:c1], kh[:, kb * P:(kb + 1) * P],
                            qT_sb[:, h, qc * TC + c0:qc * TC + c1],
                            start=True, stop=(pat is None))
                        if pat is not None:
                            nc.tensor.matmul(
                                s_ps[:, c0:c1], ident_sb[:],
                                masks_sb[:, pat, c0:c1],
                                start=False, stop=True)
                        p_sb = pexp.tile([P, TC], bf16, tag="p",
                                         name="p%d_%d_%d" % (qc, h, i))
                        nc.scalar.activation(p_sb[:, c0:c1], s_ps[:, c0:c1],
                                             AF.Exp)
                        p_tiles.append((p_sb, c0, c1))
                        last = (i == len(blocks) - 1)
                        nc.tensor.matmul(
                            out_ps[:, c0:c1],
                            v_tm[:, kb, h * P:(h + 1) * P], p_sb[:, c0:c1],
                            start=(i == 0), stop=last)
                    # den after the pv stream: ones_col stays stationary
                    for i, (p_sb, c0, c1) in enumerate(p_tiles):
                        nc.tensor.matmul(
                            den_ps[0:1, c0:c1], ones_col[:], p_sb[:, c0:c1],
                            start=(i == 0), stop=(i == len(p_tiles) - 1))
                    rec = ppool.tile([1, TC], f32r, tag="rec")
                    with nc.allow_low_precision(
                            reason="f32r reciprocal of softmax sum"):
                        nc.vector.reciprocal(rec[:], den_ps[:])
                    rb_ps = psB.tile([P, TC], f32, tag="rb")
                    nc.tensor.matmul(rb_ps[:], ones_row[:], rec[:],
                                     start=True, stop=True)
                    rb_sb = ppool.tile([P, TC], f32, tag="rb")
                    nc.vector.tensor_copy(rb_sb[:], rb_ps[:])
                    nc.vector.tensor_tensor(
                        attn_sb[:, h, qsl], out_ps[:], rb_sb[:], ALU.mult)

                # ---- output projection for this token chunk ----
                if not ABLATE.get("noproj"):
                    strip = ostg.tile([P, H // P, TC], bf16, tag="ostrip",
                                      name="strip%d" % qc)
                    for ot in range(H // P):
                        o_ps = psP.tile([P, TC], f32, tag="proj")
                        for ft in range(HPG):
                            nc.tensor.matmul(
                                o_ps[:], wo_sb[:, ft, ot * P:(ot + 1) * P],
                                attn_sb[:, ft, qsl],
                                start=(ft == 0), stop=(ft == HPG - 1))
                        if True:
                            nc.vector.tensor_copy(strip[:, ot, :], o_ps[:])
                        else:
                            nc.scalar.activation(strip[:, ot, :], o_ps[:],
                                                 AF.Copy)
                    nc.sync.dma_start(
                        out_ext.rearrange("(ot p) t -> p ot t", p=P)[:, :, qsl],
                        strip[:])


_prep_cache = {}


def _prep_in_maps(hidden_states, w_qkv, q_norm_w, k_norm_w, w_o):
    key = (id(hidden_states), id(w_qkv), id(w_o))
    if key in _prep_cache:
        return _prep_cache[key][0]

    hs = np.asarray(hidden_states, dtype=np.float32)
    wq = np.asarray(w_qkv, dtype=np.float32)
    wo = np.asarray(w_o, dtype=np.float32)
    qw = np.asarray(q_norm_w, dtype=np.float32)
    kw = np.asarray(k_norm_w, dtype=np.float32)

    hsT = [np.ascontiguousarray(hs[b].T).astype(ml_dtypes.bfloat16)
           for b in range(B)]
    in_maps = []
    for c in range(N_CORES):
        b, hg = divmod(c, HG)
        sl = slice(hg * FPG, (hg + 1) * FPG)
        wT = np.ascontiguousarray(
            np.concatenate([wq[0 * H:][sl], wq[1 * H:][sl], wq[2 * H:][sl]],
                           axis=0).T).astype(ml_dtypes.bfloat16)
        woT = np.ascontiguousarray(wo[:, sl].T).astype(ml_dtypes.bfloat16)
        in_maps.append({
            "hsT": hsT[b],
            "wT": wT,
            "woT": woT,
            "qw": np.ascontiguousarray(qw[sl]),
            "kw": np.ascontiguousarray(kw[sl]),
        })
    _prep_cache[key] = (in_maps, hidden_states, w_qkv, w_o)
    return in_maps


def kernel(hidden_states, w_qkv, q_norm_w, k_norm_w, w_o):
    qw = np.asarray(q_norm_w, dtype=np.float32)
    kw = np.asarray(k_norm_w, dtype=np.float32)
    apply_w = not (np.allclose(qw, 1.0) and np.allclose(kw, 1.0))

    nc = _build(apply_w)
    in_maps = _prep_in_maps(hidden_states, w_qkv, q_norm_w, k_norm_w, w_o)
    res = run_bass_kernel_spmd(
        nc, in_maps, core_ids=list(range(N_CORES)),
        trace=bool(int(os.environ.get("KERNEL_TRACE", "0"))))
    _cache["last_results"] = res

    out = np.zeros((B, S, H), np.float32)
    for b in range(B):
        acc = res.results[b * HG]["outT"].astype(np.float32)
        for hg in range(1, HG):
            acc += res.results[b * HG + hg]["outT"].astype(np.float32)
        out[b] = acc.T
    return out
